# revision 67
# baseline (speedup 1.0000x reference)
"""Trainium2 Bass kernel for nn_MoEGate_6150393168540 (moe_routing).

Computes, for x [B=65536, D=1024], gate/expert weights [E=8, D] and biases [E]:
    gate = softmax(x @ gate_w.T + gate_b)            # [B, 8]
    keep top-k (k=2) gate values, zero the rest (no renormalization)
    expert = x @ expert_w.T + expert_b               # [B, 8]
    out = sum(gate_masked * expert, axis=1)          # [B, 1]

Strategy (8 NeuronCores, data-parallel over the batch):
  - Each core gets 8192 rows of x; weights are replicated.
  - The D-contraction needs x transposed (D on partitions). PE transposes x in
    fp32 ([128,128] blocks -> PSUM, bit-exact).
  - Scores must order-match a pure-fp32 reference (top-2 selection), so the
    matmul uses an exact fp16 Dekker split: hi = bf16-truncate(xT),
    lo = fp16(xT - hi) (ACT extracts hi from PSUM, DVE computes lo). Weights
    are split host-side the same way. hi*w_hi, hi*w_lo, lo*w_hi are exact
    products accumulated in fp32 PSUM => scores accurate to ~1e-7 at 16-bit
    matmul speed.
  - mm_hi: lhsT=[w_hi|w_lo] [128,32] -> psum rows 0:32; mm_lo accumulates.
  - A tiny "fold" matmul (lhsT = scores [32,128-col slice], rhs = [I16;I16])
    both transposes scores back to batch-major and sums the two partial rows.
  - Postprocess per PSUM bank [128 rows, 32 groups, 16]: +bias, exp on ACT,
    top-2 threshold via a min/max tournament tree, mask, weighted sum, divide
    by the softmax denominator; DVE 32x32 block transpose makes the output
    DMA contiguous.

v2 performance changes vs v1 (baseline 181.5us):
  - x SBUF layout "(p j) d": partition p holds 4 consecutive HBM rows so each
    x-block DMA is 128 descriptors of 16KB contiguous (vs 512 x 4KB). The
    resulting row permutation within each 512-block is undone on the host.
  - const DMAs go on the scalar-engine HWDGE ring before block 0 so x DMAs
    (sync ring) start immediately and whl/ident are resident early.
  - xt PSUM tiles span 2 banks (2 contraction chunks); the hi/lo extraction
    runs once per 2 chunks on [128,1024] so ACT/DVE fixed overheads and
    semaphore round-trips halve.
v3:
  - whl is pre-transposed host-side so its DMA is 128 x 512B descriptors
    (was 1024 x 64B hogging all 16 SDMA engines at t=0 and delaying x).
  - block 0 loads in d-quarters so the first transposes start at ~1us.
  - hi extraction is a contiguous fp32->bf16 cast-copy (RNE) instead of a
    strided u16 bit-slice: ACT runs ~1.5x faster, freeing xt banks sooner.
  - postprocess granularity shrinks toward the end (8/4/2/1/1 blocks per
    group) so the serial zz+fold+postprocess tail after the last matmul is
    short; each group's output leaves in ONE dma (512B/partition) via four
    32x32 DVE transposes gathered onto shared partitions.
  - folds are deferred one block so their weight loads never stall the PE
    on the zz extraction chain.
v12/final (134.5us vs 181.5us baseline; later same-code runs measured up to
163us when the shared HBM stack was delivering only ~330 GB/s instead of
~420 — expect run-to-run environment variance):
  - the fold consumes the block's scores as a raw fp32 SBUF copy: a native
    fp32 matmul with an exact 0/1 streamed operand keeps stationary fp32
    near-exactly (measured rel err 5e-8 in micro_fp32mm.py), so the fp16
    Dekker score pair and its DVE subtract are gone and folds halve to 4.
  - top-2 threshold via max / zero-argmax / max (5 DVE ops) instead of the
    9-op min/max tournament tree.
earlier structural notes (PE col-tiling + deep PSUM pipeline):
  - the 4 hi/lo streams of 2 chunks issue as a COLUMN-TILED QUAD
    (tile_position=(0,32*cg)): each stream occupies a distinct 32-column
    group of the PE array and they run concurrently on separate XBUSes,
    so the quad costs ~1 stream-time instead of 4. z_ps is [128,512]
    (4 col-groups x [16 w_hi | 16 w_lo] partials) and the fold matrix
    [128,16] = tile(I16,(8,1)) sums all 8 score groups in one matmul.
  - PSUM: 6 banks of xt (3 tiles x 2 chunks -> extraction pipeline depth
    3, transposes never wait on ACT/DVE) + 1 z + 1 zt.
  - steady block period ~6.2us vs the 5.86us/block HBM floor (358 GB/s
    per core); DVE is the pacing engine at (N+151)/0.96ns per
    tensor_tensor. Lesson from failed variants v4/v6/v8: batched
    extraction ops beat per-bank/per-chunk splits -- the extra semaphore
    round-trips and per-op fixed costs outweigh earlier bank release.
"""

import sys

sys.path.insert(0, "/opt/trn_rl_repo")

from contextlib import ExitStack

import numpy as np

import concourse.bass as bass
import concourse.mybir as mybir
import concourse.tile as tile

F32 = mybir.dt.float32
F16 = mybir.dt.float16
BF16 = mybir.dt.bfloat16
U16 = mybir.dt.uint16
ALU = mybir.AluOpType
AXX = mybir.AxisListType.X
EXP = mybir.ActivationFunctionType.Exp

B, D, E = 65536, 1024, 8
N_CORES = 8
B_LOC = B // N_CORES  # 8192
BLK = 512  # rows per block
DC = D // 128  # 8 contraction chunks


def split_waits(nc, max_waits: int = 1) -> int:
    """walrus here allows only one semaphore wait per instruction; hoist the
    rest into preceding single-wait NOPs on the same engine (engine streams
    execute in order, so earlier waits on the same engine are equivalent)."""
    n_split = 0
    for f in nc.m.functions:
        for bb in f.blocks:
            new = []
            for inst in bb.instructions:
                si = inst.sync_info
                if si is not None and si.on_wait and len(si.on_wait) > max_waits:
                    waits = list(si.on_wait)
                    for w in waits[:-max_waits]:
                        n_split += 1
                        nop = mybir.InstNoOp(name=f"{inst.name}-ws{n_split}")
                        nop.engine = inst.engine
                        nop.sync_info = mybir.SyncInfo(on_wait=[w], on_update=[])
                        new.append(nop)
                    inst.sync_info = mybir.SyncInfo(
                        on_wait=waits[-max_waits:], on_update=list(si.on_update or [])
                    )
                new.append(inst)
            bb.instructions = new
    return n_split


def build_module(b_loc: int = B_LOC, split: bool = True):
    assert b_loc % 4096 == 0
    nc = bass.Bass()
    x = nc.dram_tensor("x", [b_loc, D], F32, kind="ExternalInput")
    whl = nc.dram_tensor("whl", [128, DC, 32], F16, kind="ExternalInput")
    brow = nc.dram_tensor("brow", [512], F32, kind="ExternalInput")
    fold = nc.dram_tensor("fold", [128, 16], F32, kind="ExternalInput")
    ident = nc.dram_tensor("ident", [128, 128], F32, kind="ExternalInput")
    y = nc.dram_tensor("y", [b_loc], F32, kind="ExternalOutput")

    n_blk = b_loc // BLK
    # postprocess group sizes (in 512-row blocks): coarse early for low
    # per-op overhead, fine at the end so the post-matmul serial tail is short
    groups = [(0, 8), (8, 4), (12, 2), (14, 1), (15, 1)]
    blk_group = {}
    for gi, (g0, gn) in enumerate(groups):
        for b in range(gn):
            blk_group[g0 + b] = (gi, g0, gn)
    tt = nc.vector.tensor_tensor

    with tile.TileContext(nc) as tc, ExitStack() as ctx:
        consts = ctx.enter_context(tc.tile_pool(name="consts", bufs=1))
        xpool = ctx.enter_context(tc.tile_pool(name="xpool", bufs=5))
        xh_pool = ctx.enter_context(tc.tile_pool(name="xh", bufs=4))
        xl_pool = ctx.enter_context(tc.tile_pool(name="xl", bufs=4))
        z32_pool = ctx.enter_context(tc.tile_pool(name="z32", bufs=3))
        pp = ctx.enter_context(tc.tile_pool(name="pp", bufs=3))
        # PSUM: 6 xt banks (3 tiles x 2 chunks: extraction pipeline depth 3,
        # transposes never wait) + 1 z + 1 zt = 8 banks
        xt_pool = ctx.enter_context(tc.tile_pool(name="xtps", bufs=3, space="PSUM"))
        z_pool = ctx.enter_context(tc.tile_pool(name="zps", bufs=1, space="PSUM"))
        zt_pool = ctx.enter_context(tc.tile_pool(name="ztps", bufs=1, space="PSUM"))

        ident_sb = consts.tile([128, 128], F32)
        whl_sb = consts.tile([128, DC, 32], F16)
        fold_sb = consts.tile([128, 16], F32)
        bias_sb = consts.tile([128, 512], F32)

        def postprocess(zt_ps, b0, G):
            # zt_ps [128, G*16] = [128 rows, G groups, 8 gate | 8 expert]
            zb = pp.tile([128, 32, 16], F32, name="zb")[:, 0:G, :]
            nc.vector.tensor_add(
                zb, zt_ps.rearrange("p (g e) -> p g e", e=16),
                bias_sb[:, 0 : G * 16].rearrange("p (g e) -> p g e", e=16),
            )
            g8 = zb[:, :, 0:8]
            y8 = zb[:, :, 8:16]
            p8 = pp.tile([128, 32, 8], F32, name="p8")[:, 0:G, :]
            nc.scalar.activation(p8, g8, EXP)
            den = pp.tile([128, 32], F32, name="den")[:, 0:G]
            nc.vector.tensor_reduce(den, p8, axis=AXX, op=ALU.add)
            # top-2 threshold: m1 = max, zero out the argmax, m2 = next max
            m1 = pp.tile([128, 32], F32, name="m1")[:, 0:G]
            nc.vector.tensor_reduce(m1, p8, axis=AXX, op=ALU.max)
            msk1 = pp.tile([128, 32, 8], F32, name="msk1")[:, 0:G, :]
            tt(msk1, p8, m1.unsqueeze(2).to_broadcast([128, G, 8]), op=ALU.is_ge)
            pm1 = pp.tile([128, 32, 8], F32, name="pm1")[:, 0:G, :]
            tt(pm1, msk1, p8, op=ALU.mult)
            p8b = pp.tile([128, 32, 8], F32, name="p8b")[:, 0:G, :]
            tt(p8b, p8, pm1, op=ALU.subtract)
            m2f = pp.tile([128, 32], F32, name="m2f")[:, 0:G]
            nc.vector.tensor_reduce(m2f, p8b, axis=AXX, op=ALU.max)
            # mask & weighted sum
            msk = pp.tile([128, 32, 8], F32, name="msk")[:, 0:G, :]
            tt(msk, p8, m2f.unsqueeze(2).to_broadcast([128, G, 8]), op=ALU.is_ge)
            pm = pp.tile([128, 32, 8], F32, name="pm")[:, 0:G, :]
            tt(pm, msk, p8, op=ALU.mult)
            prod = pp.tile([128, 32, 8], F32, name="prod")[:, 0:G, :]
            tt(prod, pm, y8, op=ALU.mult)
            num = pp.tile([128, 32], F32, name="num")[:, 0:G]
            nc.vector.tensor_reduce(num, prod, axis=AXX, op=ALU.add)
            rden = pp.tile([128, 32], F32, name="rden")[:, 0:G]
            nc.vector.reciprocal(rden, den)
            outv = pp.tile([128, 32], F32, name="outv")
            if G < 32:
                nc.vector.memset(outv[:, G:32], 0.0)
            tt(outv[:, 0:G], num, rden, op=ALU.mult)
            # 4x 32x32 block transposes gathered onto shared partitions so the
            # output leaves in ONE dma with 512B-per-partition descriptors
            tv2 = pp.tile([32, 128], F32, name="tv2")
            for a in range(4):
                nc.vector.transpose(
                    tv2[:, 32 * a : 32 * a + 32], outv[32 * a : 32 * a + 32, :]
                )
            yf = y.ap()
            # dest[q, f] = y[b0 + 128 q + f], q in 0..G, f in 0..128
            dest = bass.AP(yf.tensor, b0, [[128, G], [1, 128]])
            nc.sync.dma_start(out=dest, in_=tv2[0:G, :])

        state = {"zt_ps": None}
        fold_q = []

        def emit_folds(blk, zc):
            # zc is the block's raw fp32 scores [128 parts, 512 rows]. The
            # fold matmul runs in native fp32: the streamed fold matrix is
            # exact 0/1, and the PE holds stationary fp32 near-exactly
            # (measured rel err 5e-8), so no fp16 Dekker pair is needed.
            gi, g0, gn = blk_group[blk]
            bank_i = blk - g0
            if bank_i == 0:
                state["zt_ps"] = zt_pool.tile(
                    [128, gn * 64], F32, name="zt_ps", tag="zt_ps",
                    padded_shape=[128, 512],
                )
            zt_ps = state["zt_ps"]
            for j in range(4):
                col = (bank_i * 4 + j) * 16
                nc.tensor.matmul(
                    zt_ps[:, col : col + 16],
                    zc[:, 128 * j : 128 * j + 128],
                    fold_sb,
                    start=True,
                    stop=True,
                )
            if bank_i == gn - 1:
                postprocess(zt_ps, g0 * 512, gn * 4)

        def emit_stage(blk, g, z_ps, xh, xl):
            # emit the hi/lo matmuls for the 2 chunks of group g as a quad of
            # column-tiled matmuls: the 4 streams go to 4 distinct 32-column
            # groups of the PE array and run CONCURRENTLY (separate XBUSes),
            # so the quad costs ~1 stream-time instead of 4. Column group
            # 2*cc+l accumulates chunk parity cc, limb l across the block's
            # 4 groups; the fold sums all 8 16-column score groups.
            for cc in range(2):
                c = 2 * g + cc
                for l, rhs in ((0, xh[:, cc, :]), (1, xl[:, cc, :])):
                    cg = 2 * cc + l
                    nc.tensor.matmul(
                        z_ps[32 * cg : 32 * cg + 32, :],
                        whl_sb[:, c, :],
                        rhs,
                        start=(g == 0),
                        stop=(g == DC // 2 - 1),
                        tile_position=(0, 32 * cg),
                    )
            if g == DC // 2 - 1:
                # block tail: raw fp32 scores to SBUF for the native-fp32 fold
                zc = z32_pool.tile([128, 512], F32)
                nc.scalar.copy(zc, z_ps)
                fold_q.append((blk, zc))
                if len(fold_q) > 1:
                    emit_folds(*fold_q.pop(0))

        pending = []
        for blk in range(n_blk):
            r0 = blk * BLK
            x_blk = xpool.tile([128, 4, D], F32, name="x_blk", tag="x_blk")
            # partition p holds HBM rows r0+4p .. r0+4p+3 (16KB contiguous per
            # partition => 16KB DMA descriptors). Host undoes the permutation.
            xin = x.ap()[r0 : r0 + BLK, :].rearrange("(p j) d -> p j d", j=4)
            if blk == 0:
                # consts on the scalar-engine ring (all with per-partition
                # contiguous descriptors): x (sync ring) is not queued behind
                # them, and they are tiny so they land first
                nc.scalar.dma_start(out=ident_sb, in_=ident.ap())
                nc.scalar.dma_start(out=whl_sb, in_=whl.ap())
                nc.scalar.dma_start(out=fold_sb, in_=fold.ap())
                nc.gpsimd.dma_start(
                    out=bias_sb,
                    in_=brow.ap().unsqueeze(0).to_broadcast([128, 512]),
                )
                # block 0: first d-quarter alone (1KB descriptors, lands
                # ~1.8us so the PE starts immediately), rest as one DMA
                nc.sync.dma_start(
                    out=x_blk[:, :, 0:256], in_=xin[:, :, 0:256]
                )
                nc.sync.dma_start(
                    out=x_blk[:, :, 256:1024], in_=xin[:, :, 256:1024]
                )
                # NOTE: the head (to ~26us) runs at HAM K=4/8 (1.2 GHz;
                # transposes 417ns vs 272 warm) because PE-transpose-mode
                # does not count as PE-busy for the clock gate. A dummy-
                # matmul warm-up pulse was tried: it DID flip HAM warm 6.7us
                # earlier, but the Tile scheduler deprioritizes consumer-less
                # work and placed the pulse at t=16.5us, adding a 9us idle
                # that cancelled the gain (135.4 vs 134.9us). Would need a
                # scheduler priority hint to land the pulse at t~0.6us.
            elif blk == 1:
                # block 1 in d-halves: arrives just as the PE finishes block
                # 0. (A/B tested both states: halves win the full-speed chip
                # state by ~2.3us; whole-block wins the ~20%-downclocked
                # thermal state by ~2.9us. Tuned for the full-speed state.)
                for q in range(2):
                    nc.sync.dma_start(
                        out=x_blk[:, :, 512 * q : 512 * q + 512],
                        in_=xin[:, :, 512 * q : 512 * q + 512],
                    )
            else:
                nc.sync.dma_start(out=x_blk, in_=xin)
            z_ps = z_pool.tile([128, 512], F32)
            for g in range(DC // 2):
                xt_ps = xt_pool.tile([128, 2, 512], F32)
                if blk == 0 and g == 0:
                    # HAM warm-up: ~3.4us of real matmuls starting as soon as
                    # whl lands (~0.6us). PE-transpose-mode doesn't count as
                    # PE-busy for the HAM clock gate, so without this the
                    # whole head runs at 1.2 GHz (transposes 417ns vs 272).
                    # Writing into THIS xt tile creates a WAW dependency that
                    # pins the pulse before the first transpose -- the Tile
                    # scheduler otherwise deprioritizes consumer-less work
                    # to t~16us where it is useless.
                    wu_rhs = whl_sb.rearrange("p c w -> p (c w)")
                    for _ in range(16):
                        nc.tensor.matmul(
                            xt_ps[0:32, 0, 0:256], whl_sb[:, 0, :], wu_rhs,
                            start=True, stop=True, skip_group_check=True,
                        )
                for cc in range(2):
                    c = 2 * g + cc
                    for j in range(4):
                        nc.tensor.transpose(
                            xt_ps[:, cc, 128 * j : 128 * j + 128],
                            x_blk[:, j, 128 * c : 128 * c + 128],
                            ident_sb,
                        )
                # hi = bf16(xT) via contiguous cast-copy on ACT (psum->sbuf,
                # RNE); lo = fp16(xT - hi) on DVE. The DVE sub MUST read hi
                # from SBUF: engines have a single PSUM read port, so an op
                # reading two PSUM views is rejected (NCC_IBVF027) -- the
                # ACT->DVE chain is architecturally forced. Products against
                # the fp16 weight pair stay exact.
                xh = xh_pool.tile([128, 2, 512], BF16)
                nc.scalar.copy(xh, xt_ps)
                xl = xl_pool.tile([128, 2, 512], F16)
                nc.vector.tensor_sub(xl, xt_ps, xh)
                pending.append((blk, g, z_ps, xh, xl))
                if len(pending) > 3:
                    emit_stage(*pending.pop(0))
        for args in pending:
            if fold_q:
                emit_folds(*fold_q.pop(0))
            emit_stage(*args)
        for args in fold_q:
            emit_folds(*args)

    if split:
        split_waits(nc)
    return nc


def host_inputs(gate_w, gate_b, expert_w, expert_b):
    """Host-side prep of the small replicated tensors."""
    W = np.concatenate([gate_w, expert_w], axis=0).astype(np.float32)  # [16, D]
    WT = W.T  # [D, 16]
    w_hi = WT.astype(np.float16)
    w_lo = (WT - w_hi.astype(np.float32)).astype(np.float16)
    whl = np.empty((128, DC, 32), dtype=np.float16)
    for c in range(DC):
        whl[:, c, 0:16] = w_hi[128 * c : 128 * (c + 1), :]
        whl[:, c, 16:32] = w_lo[128 * c : 128 * (c + 1), :]
    bcat = np.concatenate([gate_b, expert_b]).astype(np.float32)  # [16]
    brow = np.tile(bcat, 32)  # [512]
    fold = np.tile(np.eye(16), (8, 1)).astype(np.float32)
    ident = np.eye(128, dtype=np.float32)
    return {"whl": whl, "brow": brow, "fold": fold, "ident": ident}


_NC_CACHE = {}


def kernel(x, gate_w, gate_b, expert_w, expert_b, k):
    assert int(k) == 2
    x = np.ascontiguousarray(np.asarray(x, dtype=np.float32))
    assert x.shape == (B, D)

    from concourse.bass_utils import run_bass_kernel_spmd

    if B_LOC not in _NC_CACHE:
        _NC_CACHE[B_LOC] = build_module(B_LOC)
    nc = _NC_CACHE[B_LOC]

    common = host_inputs(
        np.asarray(gate_w, np.float32),
        np.asarray(gate_b, np.float32),
        np.asarray(expert_w, np.float32),
        np.asarray(expert_b, np.float32),
    )
    in_maps = [
        {**common, "x": x[i * B_LOC : (i + 1) * B_LOC]} for i in range(N_CORES)
    ]
    import os

    trace = bool(os.environ.get("MOE_TRACE"))
    if trace:
        _ensure_ntff_hook()
    res = run_bass_kernel_spmd(
        nc, in_maps, core_ids=list(range(N_CORES)), trace=trace
    )
    global LAST_RESULT
    LAST_RESULT = res
    out = np.concatenate([_unpermute(r["y"]) for r in res.results])
    return out.reshape(B, 1).astype(np.float32)


def _unpermute(y_core):
    """Undo the "(p j)" row interleave: within each 512-row block, the device
    stores the value for row 4i+j (i in 0..128, j in 0..4) at position
    128j + 32a + k where i = 32a + k.  stored[128j + 32a + k] = true[128a + 4k + j]
    => true = stored.reshape(4,4,32).transpose(1,2,0) per 512-row segment."""
    return (
        y_core.reshape(-1, 4, 4, 32).transpose(0, 2, 3, 1).reshape(-1)
    )


LAST_RESULT = None


def _ensure_ntff_hook():
    """Register the axon NTFF profile hook if the antenv shim is missing
    (lets run_bass_kernel_spmd(trace=True) capture HW timing)."""
    try:
        import antenv.axon_hooks  # noqa: F401

        return
    except ImportError:
        pass
    try:
        import types

        import antenv
        from trn_agent_boot.trn_boot import _ntff_profile_via_ctypes

        mod = types.ModuleType("antenv.axon_hooks")
        _h = [None]
        mod.set_axon_ntff_profile_hook = lambda h: _h.__setitem__(0, h)
        mod.get_axon_ntff_profile_hook = lambda: _h[0]
        sys.modules["antenv.axon_hooks"] = mod
        antenv.axon_hooks = mod
        mod.set_axon_ntff_profile_hook(
            _ntff_profile_via_ctypes("/opt/axon/libaxon_pjrt.so")
        )
    except Exception as e:  # profiling is best-effort
        print(f"ntff hook setup failed: {e}")


if __name__ == "__main__":
    rng = np.random.default_rng(0)
    s = 1.0 / np.sqrt(D)
    inputs = {
        "x": rng.standard_normal((B, D), dtype=np.float32),
        "gate_w": rng.uniform(-s, s, (E, D)).astype(np.float32),
        "gate_b": rng.uniform(-s, s, E).astype(np.float32),
        "expert_w": rng.uniform(-s, s, (E, D)).astype(np.float32),
        "expert_b": rng.uniform(-s, s, E).astype(np.float32),
        "k": 2,
    }
    got = kernel(**inputs)
    print("kernel output:", got.shape, got.dtype, got[:4, 0])


# revision 71
# speedup vs baseline: 1.0443x; 1.0443x over previous
"""Trainium2 Bass kernel for nn_MoEGate_6150393168540 (moe_routing).

Computes, for x [B=65536, D=1024], gate/expert weights [E=8, D] and biases [E]:
    gate = softmax(x @ gate_w.T + gate_b)            # [B, 8]
    keep top-k (k=2) gate values, zero the rest (no renormalization)
    expert = x @ expert_w.T + expert_b               # [B, 8]
    out = sum(gate_masked * expert, axis=1)          # [B, 1]

Strategy (8 NeuronCores, data-parallel over the batch):
  - Each core gets 8192 rows of x; weights are replicated.
  - The D-contraction needs x transposed (D on partitions). PE transposes x in
    fp32 ([128,128] blocks -> PSUM, bit-exact).
  - Scores must order-match a pure-fp32 reference (top-2 selection), so the
    matmul uses an exact fp16 Dekker split: hi = bf16-truncate(xT),
    lo = fp16(xT - hi) (ACT extracts hi from PSUM, DVE computes lo). Weights
    are split host-side the same way. hi*w_hi, hi*w_lo, lo*w_hi are exact
    products accumulated in fp32 PSUM => scores accurate to ~1e-7 at 16-bit
    matmul speed.
  - mm_hi: lhsT=[w_hi|w_lo] [128,32] -> psum rows 0:32; mm_lo accumulates.
  - A tiny "fold" matmul (lhsT = scores [32,128-col slice], rhs = [I16;I16])
    both transposes scores back to batch-major and sums the two partial rows.
  - Postprocess per PSUM bank [128 rows, 32 groups, 16]: +bias, exp on ACT,
    top-2 threshold via a min/max tournament tree, mask, weighted sum, divide
    by the softmax denominator; DVE 32x32 block transpose makes the output
    DMA contiguous.

v2 performance changes vs v1 (baseline 181.5us):
  - x SBUF layout "(p j) d": partition p holds 4 consecutive HBM rows so each
    x-block DMA is 128 descriptors of 16KB contiguous (vs 512 x 4KB). The
    resulting row permutation within each 512-block is undone on the host.
  - const DMAs go on the scalar-engine HWDGE ring before block 0 so x DMAs
    (sync ring) start immediately and whl/ident are resident early.
  - xt PSUM tiles span 2 banks (2 contraction chunks); the hi/lo extraction
    runs once per 2 chunks on [128,1024] so ACT/DVE fixed overheads and
    semaphore round-trips halve.
v3:
  - whl is pre-transposed host-side so its DMA is 128 x 512B descriptors
    (was 1024 x 64B hogging all 16 SDMA engines at t=0 and delaying x).
  - block 0 loads in d-quarters so the first transposes start at ~1us.
  - hi extraction is a contiguous fp32->bf16 cast-copy (RNE) instead of a
    strided u16 bit-slice: ACT runs ~1.5x faster, freeing xt banks sooner.
  - postprocess granularity shrinks toward the end (8/4/2/1/1 blocks per
    group) so the serial zz+fold+postprocess tail after the last matmul is
    short; each group's output leaves in ONE dma (512B/partition) via four
    32x32 DVE transposes gathered onto shared partitions.
  - folds are deferred one block so their weight loads never stall the PE
    on the zz extraction chain.
v12/final (134.5us vs 181.5us baseline; later same-code runs measured up to
163us when the shared HBM stack was delivering only ~330 GB/s instead of
~420 — expect run-to-run environment variance):
  - the fold consumes the block's scores as a raw fp32 SBUF copy: a native
    fp32 matmul with an exact 0/1 streamed operand keeps stationary fp32
    near-exactly (measured rel err 5e-8 in micro_fp32mm.py), so the fp16
    Dekker score pair and its DVE subtract are gone and folds halve to 4.
  - top-2 threshold via max / zero-argmax / max (5 DVE ops) instead of the
    9-op min/max tournament tree.
earlier structural notes (PE col-tiling + deep PSUM pipeline):
  - the 4 hi/lo streams of 2 chunks issue as a COLUMN-TILED QUAD
    (tile_position=(0,32*cg)): each stream occupies a distinct 32-column
    group of the PE array and they run concurrently on separate XBUSes,
    so the quad costs ~1 stream-time instead of 4. z_ps is [128,512]
    (4 col-groups x [16 w_hi | 16 w_lo] partials) and the fold matrix
    [128,16] = tile(I16,(8,1)) sums all 8 score groups in one matmul.
  - PSUM: 6 banks of xt (3 tiles x 2 chunks -> extraction pipeline depth
    3, transposes never wait on ACT/DVE) + 1 z + 1 zt.
  - steady block period ~6.2us vs the 5.86us/block HBM floor (358 GB/s
    per core); DVE is the pacing engine at (N+151)/0.96ns per
    tensor_tensor. Lesson from failed variants v4/v6/v8: batched
    extraction ops beat per-bank/per-chunk splits -- the extra semaphore
    round-trips and per-op fixed costs outweigh earlier bank release.
"""

import sys

sys.path.insert(0, "/opt/trn_rl_repo")

from contextlib import ExitStack

import numpy as np

import concourse.bass as bass
import concourse.mybir as mybir
import concourse.tile as tile

F32 = mybir.dt.float32
F16 = mybir.dt.float16
BF16 = mybir.dt.bfloat16
U16 = mybir.dt.uint16
ALU = mybir.AluOpType
AXX = mybir.AxisListType.X
EXP = mybir.ActivationFunctionType.Exp

B, D, E = 65536, 1024, 8
N_CORES = 8
B_LOC = B // N_CORES  # 8192
BLK = 512  # rows per block
DC = D // 128  # 8 contraction chunks


def split_waits(nc, max_waits: int = 1) -> int:
    """walrus here allows only one semaphore wait per instruction; hoist the
    rest into preceding single-wait NOPs on the same engine (engine streams
    execute in order, so earlier waits on the same engine are equivalent)."""
    n_split = 0
    for f in nc.m.functions:
        for bb in f.blocks:
            new = []
            for inst in bb.instructions:
                si = inst.sync_info
                if si is not None and si.on_wait and len(si.on_wait) > max_waits:
                    waits = list(si.on_wait)
                    for w in waits[:-max_waits]:
                        n_split += 1
                        nop = mybir.InstNoOp(name=f"{inst.name}-ws{n_split}")
                        nop.engine = inst.engine
                        nop.sync_info = mybir.SyncInfo(on_wait=[w], on_update=[])
                        new.append(nop)
                    inst.sync_info = mybir.SyncInfo(
                        on_wait=waits[-max_waits:], on_update=list(si.on_update or [])
                    )
                new.append(inst)
            bb.instructions = new
    return n_split


def build_module(b_loc: int = B_LOC, split: bool = True):
    assert b_loc % 4096 == 0
    nc = bass.Bass()
    x = nc.dram_tensor("x", [b_loc, D], F32, kind="ExternalInput")
    whl = nc.dram_tensor("whl", [128, DC, 32], F16, kind="ExternalInput")
    brow = nc.dram_tensor("brow", [512], F32, kind="ExternalInput")
    fold = nc.dram_tensor("fold", [128, 16], F32, kind="ExternalInput")
    ident = nc.dram_tensor("ident", [128, 128], F32, kind="ExternalInput")
    y = nc.dram_tensor("y", [b_loc], F32, kind="ExternalOutput")

    n_blk = b_loc // BLK
    # postprocess group sizes (in 512-row blocks): coarse early for low
    # per-op overhead, fine at the end so the post-matmul serial tail is short
    groups = [(0, 8), (8, 4), (12, 2), (14, 1), (15, 1)]
    blk_group = {}
    for gi, (g0, gn) in enumerate(groups):
        for b in range(gn):
            blk_group[g0 + b] = (gi, g0, gn)
    tt = nc.vector.tensor_tensor

    with tile.TileContext(nc) as tc, ExitStack() as ctx:
        consts = ctx.enter_context(tc.tile_pool(name="consts", bufs=1))
        xpool = ctx.enter_context(tc.tile_pool(name="xpool", bufs=5))
        xh_pool = ctx.enter_context(tc.tile_pool(name="xh", bufs=4))
        xl_pool = ctx.enter_context(tc.tile_pool(name="xl", bufs=4))
        z32_pool = ctx.enter_context(tc.tile_pool(name="z32", bufs=3))
        pp = ctx.enter_context(tc.tile_pool(name="pp", bufs=3))
        # PSUM: 6 xt banks (3 tiles x 2 chunks: extraction pipeline depth 3,
        # transposes never wait) + 1 z + 1 zt = 8 banks
        xt_pool = ctx.enter_context(tc.tile_pool(name="xtps", bufs=3, space="PSUM"))
        z_pool = ctx.enter_context(tc.tile_pool(name="zps", bufs=1, space="PSUM"))
        zt_pool = ctx.enter_context(tc.tile_pool(name="ztps", bufs=1, space="PSUM"))

        ident_sb = consts.tile([128, 128], F32)
        whl_sb = consts.tile([128, DC, 32], F16)
        fold_sb = consts.tile([128, 16], F32)
        bias_sb = consts.tile([128, 512], F32)
        # identity recomputed on the PE by the HAM warm-up pulse; the
        # transposes read THIS copy, making the pulse a true RAW dependency
        ident2_sb = consts.tile([128, 128], F32)
        zero_sb = consts.tile([128, 128], F32)

        def postprocess(zt_ps, b0, G):
            # zt_ps [128, G*16] = [128 rows, G groups, 8 gate | 8 expert]
            zb = pp.tile([128, 32, 16], F32, name="zb")[:, 0:G, :]
            nc.vector.tensor_add(
                zb, zt_ps.rearrange("p (g e) -> p g e", e=16),
                bias_sb[:, 0 : G * 16].rearrange("p (g e) -> p g e", e=16),
            )
            g8 = zb[:, :, 0:8]
            y8 = zb[:, :, 8:16]
            p8 = pp.tile([128, 32, 8], F32, name="p8")[:, 0:G, :]
            nc.scalar.activation(p8, g8, EXP)
            den = pp.tile([128, 32], F32, name="den")[:, 0:G]
            nc.vector.tensor_reduce(den, p8, axis=AXX, op=ALU.add)
            # top-2 threshold: m1 = max, zero out the argmax, m2 = next max
            m1 = pp.tile([128, 32], F32, name="m1")[:, 0:G]
            nc.vector.tensor_reduce(m1, p8, axis=AXX, op=ALU.max)
            msk1 = pp.tile([128, 32, 8], F32, name="msk1")[:, 0:G, :]
            tt(msk1, p8, m1.unsqueeze(2).to_broadcast([128, G, 8]), op=ALU.is_ge)
            pm1 = pp.tile([128, 32, 8], F32, name="pm1")[:, 0:G, :]
            tt(pm1, msk1, p8, op=ALU.mult)
            p8b = pp.tile([128, 32, 8], F32, name="p8b")[:, 0:G, :]
            tt(p8b, p8, pm1, op=ALU.subtract)
            m2f = pp.tile([128, 32], F32, name="m2f")[:, 0:G]
            nc.vector.tensor_reduce(m2f, p8b, axis=AXX, op=ALU.max)
            # mask & weighted sum
            msk = pp.tile([128, 32, 8], F32, name="msk")[:, 0:G, :]
            tt(msk, p8, m2f.unsqueeze(2).to_broadcast([128, G, 8]), op=ALU.is_ge)
            pm = pp.tile([128, 32, 8], F32, name="pm")[:, 0:G, :]
            tt(pm, msk, p8, op=ALU.mult)
            prod = pp.tile([128, 32, 8], F32, name="prod")[:, 0:G, :]
            tt(prod, pm, y8, op=ALU.mult)
            num = pp.tile([128, 32], F32, name="num")[:, 0:G]
            nc.vector.tensor_reduce(num, prod, axis=AXX, op=ALU.add)
            rden = pp.tile([128, 32], F32, name="rden")[:, 0:G]
            nc.vector.reciprocal(rden, den)
            outv = pp.tile([128, 32], F32, name="outv")
            if G < 32:
                nc.vector.memset(outv[:, G:32], 0.0)
            tt(outv[:, 0:G], num, rden, op=ALU.mult)
            # 4x 32x32 block transposes gathered onto shared partitions so the
            # output leaves in ONE dma with 512B-per-partition descriptors
            tv2 = pp.tile([32, 128], F32, name="tv2")
            for a in range(4):
                nc.vector.transpose(
                    tv2[:, 32 * a : 32 * a + 32], outv[32 * a : 32 * a + 32, :]
                )
            yf = y.ap()
            # dest[q, f] = y[b0 + 128 q + f], q in 0..G, f in 0..128
            dest = bass.AP(yf.tensor, b0, [[128, G], [1, 128]])
            nc.sync.dma_start(out=dest, in_=tv2[0:G, :])

        state = {"zt_ps": None}
        fold_q = []

        def emit_folds(blk, zc):
            # zc is the block's raw fp32 scores [128 parts, 512 rows]. The
            # fold matmul runs in native fp32: the streamed fold matrix is
            # exact 0/1, and the PE holds stationary fp32 near-exactly
            # (measured rel err 5e-8), so no fp16 Dekker pair is needed.
            gi, g0, gn = blk_group[blk]
            bank_i = blk - g0
            if bank_i == 0:
                state["zt_ps"] = zt_pool.tile(
                    [128, gn * 64], F32, name="zt_ps", tag="zt_ps",
                    padded_shape=[128, 512],
                )
            zt_ps = state["zt_ps"]
            for j in range(4):
                col = (bank_i * 4 + j) * 16
                nc.tensor.matmul(
                    zt_ps[:, col : col + 16],
                    zc[:, 128 * j : 128 * j + 128],
                    fold_sb,
                    start=True,
                    stop=True,
                )
            if bank_i == gn - 1:
                postprocess(zt_ps, g0 * 512, gn * 4)

        def emit_stage(blk, g, z_ps, xh, xl):
            # emit the hi/lo matmuls for the 2 chunks of group g as a quad of
            # column-tiled matmuls: the 4 streams go to 4 distinct 32-column
            # groups of the PE array and run CONCURRENTLY (separate XBUSes),
            # so the quad costs ~1 stream-time instead of 4. Column group
            # 2*cc+l accumulates chunk parity cc, limb l across the block's
            # 4 groups; the fold sums all 8 16-column score groups.
            for cc in range(2):
                c = 2 * g + cc
                for l, rhs in ((0, xh[:, cc, :]), (1, xl[:, cc, :])):
                    cg = 2 * cc + l
                    nc.tensor.matmul(
                        z_ps[32 * cg : 32 * cg + 32, :],
                        whl_sb[:, c, :],
                        rhs,
                        start=(g == 0),
                        stop=(g == DC // 2 - 1),
                        tile_position=(0, 32 * cg),
                    )
            if g == DC // 2 - 1:
                # block tail: raw fp32 scores to SBUF for the native-fp32 fold
                zc = z32_pool.tile([128, 512], F32)
                nc.scalar.copy(zc, z_ps)
                fold_q.append((blk, zc))
                if len(fold_q) > 1:
                    emit_folds(*fold_q.pop(0))

        pending = []
        for blk in range(n_blk):
            r0 = blk * BLK
            x_blk = xpool.tile([128, 4, D], F32, name="x_blk", tag="x_blk")
            # partition p holds HBM rows r0+4p .. r0+4p+3 (16KB contiguous per
            # partition => 16KB DMA descriptors). Host undoes the permutation.
            xin = x.ap()[r0 : r0 + BLK, :].rearrange("(p j) d -> p j d", j=4)
            if blk == 0:
                # consts on the scalar-engine ring (all with per-partition
                # contiguous descriptors): x (sync ring) is not queued behind
                # them, and they are tiny so they land first
                nc.scalar.dma_start(out=ident_sb, in_=ident.ap())
                nc.scalar.dma_start(out=whl_sb, in_=whl.ap())
                nc.scalar.dma_start(out=fold_sb, in_=fold.ap())
                nc.gpsimd.dma_start(
                    out=bias_sb,
                    in_=brow.ap().unsqueeze(0).to_broadcast([128, 512]),
                )
                # block 0: first d-quarter alone (1KB descriptors, lands
                # ~1.8us so the PE starts immediately), rest as one DMA
                nc.sync.dma_start(
                    out=x_blk[:, :, 0:256], in_=xin[:, :, 0:256]
                )
                nc.sync.dma_start(
                    out=x_blk[:, :, 256:1024], in_=xin[:, :, 256:1024]
                )
                # HAM warm-up: the head otherwise runs at K=4/8 (1.2 GHz;
                # transposes 417ns vs 272 warm) until ~26us because
                # PE-transpose-mode does not count as PE-busy for the clock
                # gate. The pulse below is ~3.4us of real matmuls that
                # COMPUTE the identity the transposes read (I.T @ I plus 15
                # exact zero-adds in one ordered accumulation group): a true
                # RAW dependency, so the Tile scheduler cannot defer it the
                # way it deferred consumer-less dummy pulses (to t~16-19us,
                # twice). It starts as soon as ident lands (~0.6us), in
                # otherwise-idle DMA-ramp time.
                nc.vector.memset(zero_sb, 0.0)
                wu_ps = zt_pool.tile([128, 512], F32, name="wu_ps", tag="zt_ps")
                for i in range(16):
                    nc.tensor.matmul(
                        wu_ps[:, 0:128],
                        ident_sb,
                        ident_sb if i == 0 else zero_sb,
                        start=(i == 0),
                        stop=(i == 15),
                    )
                nc.scalar.copy(ident2_sb, wu_ps[:, 0:128])
            elif blk == 1:
                # block 1 in d-halves: arrives just as the PE finishes block
                # 0. (A/B tested both states: halves win the full-speed chip
                # state by ~2.3us; whole-block wins the ~20%-downclocked
                # thermal state by ~2.9us. Tuned for the full-speed state.)
                for q in range(2):
                    nc.sync.dma_start(
                        out=x_blk[:, :, 512 * q : 512 * q + 512],
                        in_=xin[:, :, 512 * q : 512 * q + 512],
                    )
            else:
                nc.sync.dma_start(out=x_blk, in_=xin)
            z_ps = z_pool.tile([128, 512], F32)
            for g in range(DC // 2):
                xt_ps = xt_pool.tile([128, 2, 512], F32)
                for cc in range(2):
                    c = 2 * g + cc
                    for j in range(4):
                        nc.tensor.transpose(
                            xt_ps[:, cc, 128 * j : 128 * j + 128],
                            x_blk[:, j, 128 * c : 128 * c + 128],
                            ident2_sb,
                        )
                # hi = bf16(xT) via contiguous cast-copy on ACT (psum->sbuf,
                # RNE); lo = fp16(xT - hi) on DVE. The DVE sub MUST read hi
                # from SBUF: engines have a single PSUM read port, so an op
                # reading two PSUM views is rejected (NCC_IBVF027) -- the
                # ACT->DVE chain is architecturally forced. Products against
                # the fp16 weight pair stay exact.
                xh = xh_pool.tile([128, 2, 512], BF16)
                nc.scalar.copy(xh, xt_ps)
                xl = xl_pool.tile([128, 2, 512], F16)
                nc.vector.tensor_sub(xl, xt_ps, xh)
                pending.append((blk, g, z_ps, xh, xl))
                if len(pending) > 3:
                    emit_stage(*pending.pop(0))
        for args in pending:
            if fold_q:
                emit_folds(*fold_q.pop(0))
            emit_stage(*args)
        for args in fold_q:
            emit_folds(*args)

    if split:
        split_waits(nc)
    return nc


def host_inputs(gate_w, gate_b, expert_w, expert_b):
    """Host-side prep of the small replicated tensors."""
    W = np.concatenate([gate_w, expert_w], axis=0).astype(np.float32)  # [16, D]
    WT = W.T  # [D, 16]
    w_hi = WT.astype(np.float16)
    w_lo = (WT - w_hi.astype(np.float32)).astype(np.float16)
    whl = np.empty((128, DC, 32), dtype=np.float16)
    for c in range(DC):
        whl[:, c, 0:16] = w_hi[128 * c : 128 * (c + 1), :]
        whl[:, c, 16:32] = w_lo[128 * c : 128 * (c + 1), :]
    bcat = np.concatenate([gate_b, expert_b]).astype(np.float32)  # [16]
    brow = np.tile(bcat, 32)  # [512]
    fold = np.tile(np.eye(16), (8, 1)).astype(np.float32)
    ident = np.eye(128, dtype=np.float32)
    return {"whl": whl, "brow": brow, "fold": fold, "ident": ident}


_NC_CACHE = {}


def kernel(x, gate_w, gate_b, expert_w, expert_b, k):
    assert int(k) == 2
    x = np.ascontiguousarray(np.asarray(x, dtype=np.float32))
    assert x.shape == (B, D)

    from concourse.bass_utils import run_bass_kernel_spmd

    if B_LOC not in _NC_CACHE:
        _NC_CACHE[B_LOC] = build_module(B_LOC)
    nc = _NC_CACHE[B_LOC]

    common = host_inputs(
        np.asarray(gate_w, np.float32),
        np.asarray(gate_b, np.float32),
        np.asarray(expert_w, np.float32),
        np.asarray(expert_b, np.float32),
    )
    in_maps = [
        {**common, "x": x[i * B_LOC : (i + 1) * B_LOC]} for i in range(N_CORES)
    ]
    import os

    trace = bool(os.environ.get("MOE_TRACE"))
    if trace:
        _ensure_ntff_hook()
    res = run_bass_kernel_spmd(
        nc, in_maps, core_ids=list(range(N_CORES)), trace=trace
    )
    global LAST_RESULT
    LAST_RESULT = res
    out = np.concatenate([_unpermute(r["y"]) for r in res.results])
    return out.reshape(B, 1).astype(np.float32)


def _unpermute(y_core):
    """Undo the "(p j)" row interleave: within each 512-row block, the device
    stores the value for row 4i+j (i in 0..128, j in 0..4) at position
    128j + 32a + k where i = 32a + k.  stored[128j + 32a + k] = true[128a + 4k + j]
    => true = stored.reshape(4,4,32).transpose(1,2,0) per 512-row segment."""
    return (
        y_core.reshape(-1, 4, 4, 32).transpose(0, 2, 3, 1).reshape(-1)
    )


LAST_RESULT = None


def _ensure_ntff_hook():
    """Register the axon NTFF profile hook if the antenv shim is missing
    (lets run_bass_kernel_spmd(trace=True) capture HW timing)."""
    try:
        import antenv.axon_hooks  # noqa: F401

        return
    except ImportError:
        pass
    try:
        import types

        import antenv
        from trn_agent_boot.trn_boot import _ntff_profile_via_ctypes

        mod = types.ModuleType("antenv.axon_hooks")
        _h = [None]
        mod.set_axon_ntff_profile_hook = lambda h: _h.__setitem__(0, h)
        mod.get_axon_ntff_profile_hook = lambda: _h[0]
        sys.modules["antenv.axon_hooks"] = mod
        antenv.axon_hooks = mod
        mod.set_axon_ntff_profile_hook(
            _ntff_profile_via_ctypes("/opt/axon/libaxon_pjrt.so")
        )
    except Exception as e:  # profiling is best-effort
        print(f"ntff hook setup failed: {e}")


if __name__ == "__main__":
    rng = np.random.default_rng(0)
    s = 1.0 / np.sqrt(D)
    inputs = {
        "x": rng.standard_normal((B, D), dtype=np.float32),
        "gate_w": rng.uniform(-s, s, (E, D)).astype(np.float32),
        "gate_b": rng.uniform(-s, s, E).astype(np.float32),
        "expert_w": rng.uniform(-s, s, (E, D)).astype(np.float32),
        "expert_b": rng.uniform(-s, s, E).astype(np.float32),
        "k": 2,
    }
    got = kernel(**inputs)
    print("kernel output:", got.shape, got.dtype, got[:4, 0])
